# revision 18
# baseline (speedup 1.0000x reference)
"""Multi-head attention forward (B=8, N=1024, C=768, H=12) on 8 TRN2 NeuronCores.

Sharding: data-parallel over batch — core b computes batch b end-to-end
(weights replicated, no collectives). Per-core dataflow, all matmuls bf16
with fp32 PSUM accumulation:

  x [1024,768] --cast+PE transpose (batched per seq block)--> xT [768,1024]
  qT,kT[t] = w_qkv-stationary matmuls over xT               [feat, seq]
  v        = xT-stationary matmuls over w_qkv[:, 1536:]     [seq, feat(+ones)]
  S^T  = kT-stationary matmuls over qT (2 heads packed in PE row groups)
  E^T  = exp(S^T / 8) via ACT straight from PSUM (no max-sub needed)
  PV   = v_aug-stationary matmuls over E^T -> [out^T ; rowsum] in PSUM
  out^T = PV[0:64] * (1/rowsum)   (gpsimd partition-broadcast + fast recip)
  y = out^T-stationary matmuls over w_proj + bias, stored bf16

Schedule (trace-driven): the PE is the bottleneck engine (~166us busy of
~198us wall; fp8 double-pumping would halve the PV cost but e4m3
quantization of E or v adds ~2.2-3.1e-2 max-rel error vs the 2e-2 budget,
measured in simulation, so everything stays bf16). What remains is keeping
the PE stream dense:
  - 28 dummy identity transposes spin the PE up to full p-state while the
    first x block is still in flight (the PE runs at half clock for ~3us
    after any idle period).
  - x arrives as 8 per-seq-block descriptors on the SP hardware DMA queue,
    depth-4 dependency-chained so blocks complete in order (unchained
    concurrent descriptors all complete striped-at-the-end, which starves
    the transposes); q+k weights ride the ACT queue as 6 concurrent
    full-row descriptors; v weights + w_proj follow on SP. The nh=0 q/k
    matmul groups are woven between transpose batches to cover the x tail.
  - per-seq-block x^T psum is drained by a single DVE copy (bf16 2x mode)
    into one [128, CB, N] tile, keeping ACT free of prologue work.
  - v-computation is woven into the first head-pair's attention between
    exp and PV; the next head-pair's q/k matmuls are woven one psum-group
    per jp into the previous pair's second i-half so the ACT exp queue
    never drains at head boundaries.
  - proj epilogue: ACT drains each py psum half (fast psum recycle while
    DVE finishes the last normalizations), DVE adds the bias in SBUF off
    the recycle path, y is stored bf16 per half-block (host upcasts).
Measured ~197-200us per core (run-to-run thermal throttle noise ~±4us),
output max rel err 8.6e-3 vs the fp32 reference.
"""
import numpy as np
from contextlib import ExitStack

import concourse.bacc as bacc
import concourse.tile as tile
from concourse import mybir, bass_utils, masks
from concourse.tile import add_dep_helper

F32 = mybir.dt.float32
BF16 = mybir.dt.bfloat16
EXP = mybir.ActivationFunctionType.Exp
MUL = mybir.AluOpType.mult
ADD = mybir.AluOpType.add

QKV_DT = BF16
ATT_DT = BF16
PROJ_DT = BF16

B = 8
N = 1024       # sequence length
C = 768        # channels
H = 12         # heads
HD = 64        # head dim
NB = N // 128  # 8 seq blocks
CB = C // 128  # 6 channel chunks
HP = H // 2    # 6 head pairs
VW = HD + 1    # 65: v columns per head incl. ones column
SCALE = float(HD) ** -0.5

_NC = None


def _build():
    nc = bacc.Bacc("TRN2", target_bir_lowering=False, debug=False, num_devices=B)
    x = nc.dram_tensor("x", [N, C], BF16, kind="ExternalInput")
    w_qkv = nc.dram_tensor("w_qkv", [C, 3 * C], BF16, kind="ExternalInput")
    w_proj = nc.dram_tensor("w_proj", [C, C], BF16, kind="ExternalInput")
    b_proj = nc.dram_tensor("b_proj", [1, C], F32, kind="ExternalInput")
    y = nc.dram_tensor("y", [N, C], BF16, kind="ExternalOutput")

    with tile.TileContext(nc) as tc, ExitStack() as ctx:
        const = ctx.enter_context(tc.tile_pool(name="const", bufs=1))
        p_qk = ctx.enter_context(tc.tile_pool(name="p_qk", bufs=1))
        p_v = ctx.enter_context(tc.tile_pool(name="p_v", bufs=1))
        p_out = ctx.enter_context(tc.tile_pool(name="p_out", bufs=1))

        # ---- whole-tensor input tiles. Many concurrent descriptors across
        # BOTH hardware DMA queues (SP + ACT): the queue sprays concurrent
        # descriptors over its DMA engines, so per-queue bandwidth scales
        # with outstanding descriptors. w_qkv loads full 4608B rows per
        # packet (one descriptor per 128-row chunk); x is one descriptor
        # per seq block so transposes start on the first arrival.
        xin = const.tile([128, NB, C], BF16, tag="xin")
        wqkv = const.tile([128, CB, 3 * C], QKV_DT, tag="wqkv")
        wp = const.tile([128, CB, C], PROJ_DT, tag="wp")
        bias_row = const.tile([1, C], F32, tag="bias_row")

        x_r = x.ap().rearrange("(i p) c -> p i c", p=128)
        w_r = w_qkv.ap().rearrange("(s p) f -> p s f", p=128)
        wp_r = w_proj.ap().rearrange("(s p) f -> p s f", p=128)
        # x: SP queue, depth-2 chained so seq blocks complete in order and
        # transposes start immediately (unchained concurrent descriptors all
        # complete striped-at-the-end, which stalls the whole prologue).
        xdma = []
        for ib in range(NB):
            dma = nc.sync.dma_start(xin[:, ib:ib + 1, :], x_r[:, ib:ib + 1, :])
            if ib >= 4:
                add_dep_helper(dma.ins, xdma[ib - 4].ins, sync=True,
                               reason="x dma depth-4 chain")
            xdma.append(dma)
        # q+k band: ACT queue, 6 concurrent descriptors (3072B rows). The
        # ACT stream only issues these 6 and is then free for exp.
        for cc in range(CB):
            nc.scalar.dma_start(wqkv[:, cc:cc + 1, 0:2 * C],
                                w_r[:, cc:cc + 1, 0:2 * C])
        nc.scalar.dma_start(bias_row[:], b_proj.ap())
        # v band + w_proj: SP queue after the x chain.
        for cc in range(CB):
            nc.sync.dma_start(wqkv[:, cc:cc + 1, 2 * C:3 * C],
                              w_r[:, cc:cc + 1, 2 * C:3 * C])
        for cc2 in range(0, CB, 2):
            nc.sync.dma_start(wp[:, cc2:cc2 + 2, :], wp_r[:, cc2:cc2 + 2, :])

        ident_bf = const.tile([128, 128], BF16, tag="ident_bf")
        masks.make_identity(nc, ident_bf[:])
        bias_bc = const.tile([128, C], F32, tag="bias_bc")
        nc.gpsimd.partition_broadcast(bias_bc[:], bias_row[:])
        ones12 = const.tile([128, H], F32, tag="ones12")
        nc.vector.memset(ones12[:], 1.0)

        qT = [p_qk.tile([128, N], ATT_DT, tag=f"qT{t}", name=f"qT{t}") for t in range(HP)]
        kT = [p_qk.tile([128, N], ATT_DT, tag=f"kT{t}", name=f"kT{t}") for t in range(HP)]
        vn = [p_v.tile([128, H * VW], ATT_DT, tag=f"v{ib}", name=f"v{ib}") for ib in range(NB)]
        outT = [p_out.tile([128, N], PROJ_DT, tag=f"outT{t}", name=f"outT{t}") for t in range(HP)]

        with (
            tc.tile_pool(name="p_xT", bufs=1) as p_xT,
            tc.tile_pool(name="p_E", bufs=7) as p_E,
            tc.tile_pool(name="p_nrm", bufs=2) as p_nrm,
            tc.tile_pool(name="p_y", bufs=4) as p_y,
            tc.tile_pool(name="ps_mm", bufs=2, space="PSUM") as ps_mm,
            tc.tile_pool(name="ps_s", bufs=2, space="PSUM") as ps_s,
            tc.tile_pool(name="ps_pv", bufs=1, space="PSUM") as ps_pv,
        ):
            # ---- PE p-state warmup: dummy transposes while the first x
            # block is still in flight. The PE needs ~3us of continuous
            # execution to reach full clock; without this the first real
            # transposes+matmuls run at half speed.
            for wi in range(28):
                warm = ps_mm.tile([128, 128], BF16, tag="pmm", name=f"warm{wi}")
                nc.tensor.transpose(warm[:], ident_bf[:], ident_bf[:])

            # ---- x^T via PE transposes; one DVE copy drains the whole
            # psum block per seq chunk (bf16 2x mode), keeping ACT free
            xT = p_xT.tile([128, CB, N], QKV_DT, tag="xT", name="xT")

            def emit_transpose(ib):
                pt = ps_mm.tile([128, C], BF16, tag="pmm", name=f"ptr{ib}")
                for cc in range(CB):
                    nc.tensor.transpose(pt[:, cc * 128:(cc + 1) * 128],
                                        xin[:, ib, cc * 128:(cc + 1) * 128],
                                        ident_bf[:])
                nc.vector.tensor_copy(
                    xT[:, :, ib * 128:(ib + 1) * 128],
                    pt[:].rearrange("p (s f) -> p s f", f=128))

            def emit_qk_group(t, gi):
                f_off, dst = ((0, qT), (C, kT))[gi // 2]
                nh = gi % 2
                pq = ps_mm.tile([128, 512], F32, tag="pmm", name=f"pq{t}_{gi}")
                for cc in range(CB):
                    nc.tensor.matmul(
                        pq[:],
                        wqkv[:, cc, f_off + t * 128: f_off + (t + 1) * 128],
                        xT[:, cc, nh * 512:(nh + 1) * 512],
                        start=(cc == 0), stop=(cc == CB - 1))
                nc.vector.tensor_copy(dst[t][:, nh * 512:(nh + 1) * 512], pq[:])

            def emit_qk(t):
                for gi in range(4):
                    emit_qk_group(t, gi)

            def emit_v_group(ib, half):
                pv = ps_mm.tile([128, 384], F32, tag="pmm", name=f"pv{ib}_{half}")
                for cc in range(CB):
                    nc.tensor.matmul(
                        pv[:],
                        xT[:, cc, ib * 128:(ib + 1) * 128],
                        wqkv[:, cc, 2 * C + half * 384:2 * C + (half + 1) * 384],
                        start=(cc == 0), stop=(cc == CB - 1))
                nc.vector.tensor_copy(
                    vn[ib][:, half * 6 * VW:(half + 1) * 6 * VW]
                    .rearrange("p (h d) -> p h d", d=VW)[:, :, 0:HD],
                    pv[:].rearrange("p (h d) -> p h d", d=HD))
                if half == 1:
                    nc.vector.tensor_copy(
                        vn[ib][:].rearrange("p (h d) -> p h d", d=VW)[:, :, HD:VW],
                        ones12[:])

            # seq blocks 0-3, then the nh=0 q/k groups (they only need the
            # first half of x^T), then blocks 4-7, then the nh=1 groups —
            # keeps the PE busy while the x dma chain streams in.
            for ib in range(4):
                emit_transpose(ib)
            emit_qk_group(0, 0)
            emit_transpose(4)
            emit_qk_group(0, 2)
            for ib in range(5, NB):
                emit_transpose(ib)
            emit_qk_group(0, 1)
            emit_qk_group(0, 3)
            for ib0 in range(2):
                for h0 in range(2):
                    emit_v_group(ib0, h0)

            # proj half-block: matmul group + ACT drain + DVE bias-add +
            # store (used in the tail, and woven into the last pair's
            # exp-wait slots for nb=0)
            ys_tiles = {}

            def emit_proj_half(nb, cp, last=False):
                if nb not in ys_tiles:
                    ys_tiles[nb] = p_y.tile([128, C], BF16, tag="ys", name=f"ys{nb}")
                ys = ys_tiles[nb]
                py = ps_mm.tile([128, 384], F32, tag="pmm", name=f"py{nb}{cp}")
                for t2 in range(CB):
                    nc.tensor.matmul(
                        py[:], outT[t2][:, nb * 128:(nb + 1) * 128],
                        wp[:, t2, cp * 384:(cp + 1) * 384],
                        start=(t2 == 0), stop=(t2 == CB - 1))
                if last:
                    nc.vector.scalar_tensor_tensor(
                        ys[:, cp * 384:(cp + 1) * 384], py[:], 1.0,
                        bias_bc[:, cp * 384:(cp + 1) * 384], MUL, ADD)
                else:
                    nc.scalar.copy(ys[:, cp * 384:(cp + 1) * 384], py[:])
                    nc.vector.scalar_tensor_tensor(
                        ys[:, cp * 384:(cp + 1) * 384],
                        ys[:, cp * 384:(cp + 1) * 384], 1.0,
                        bias_bc[:, cp * 384:(cp + 1) * 384], MUL, ADD)
                nc.sync.dma_start(
                    y.ap()[nb * 128:(nb + 1) * 128, cp * 384:(cp + 1) * 384],
                    ys[:, cp * 384:(cp + 1) * 384])

            # ---- per head-pair: q^T,k^T then attention (pipelines across t)
            for t in range(HP):
                hA, hB = 2 * t, 2 * t + 1
                for ih in range(2):
                    pre_pv = None
                    if t == 0 and ih == 0:
                        pre_pv = [
                            [lambda ib=ib, h=h: emit_v_group(ib, h)
                             for ib in (2 * jp_ + 2, 2 * jp_ + 3) for h in (0, 1)]
                            for jp_ in range(3)
                        ]
                    elif ih == 1 and t + 1 < HP:
                        # weave next head-pair's q/k matmuls one psum-group per
                        # jp so the ACT exp queue never drains at the boundary
                        pre_pv = [[lambda g=g, tt=t: emit_qk_group(tt + 1, g)]
                                  for g in range(4)]
                    ppA = ps_pv.tile([VW, 512], F32, tag="pvA", name=f"ppA{t}{ih}")
                    ppB = ps_pv.tile([VW, 512], F32, tag="pvB", name=f"ppB{t}{ih}")
                    for jp in range(4):
                        jbs = (2 * jp, 2 * jp + 1)
                        sA = ps_s.tile([128, 1024], F32, tag="s2", name=f"sA{t}{ih}{jp}")
                        sB = ps_s.tile([128, 1024], F32, tag="s2", name=f"sB{t}{ih}{jp}")
                        # batch same-PE-tile-config matmuls: each (0,0) ->
                        # (64,0) row-tile reconfig costs ~90ns on hw
                        for jb, co in zip(jbs, (0, 512)):
                            nc.tensor.matmul(
                                sA[:, co:co + 512],
                                kT[t][0:64, jb * 128:(jb + 1) * 128],
                                qT[t][0:64, ih * 512:(ih + 1) * 512],
                                start=True, stop=True, tile_position=(0, 0))
                        for jb, co in zip(jbs, (0, 512)):
                            nc.tensor.matmul(
                                sB[:, co:co + 512],
                                kT[t][64:128, jb * 128:(jb + 1) * 128],
                                qT[t][64:128, ih * 512:(ih + 1) * 512],
                                start=True, stop=True, tile_position=(64, 0))
                        eA = p_E.tile([128, 1024], ATT_DT, tag="e2", name=f"eA{t}{ih}{jp}")
                        eB = p_E.tile([128, 1024], ATT_DT, tag="e2", name=f"eB{t}{ih}{jp}")
                        nc.scalar.activation(eA[:], sA[:], EXP, scale=SCALE)
                        nc.scalar.activation(eB[:], sB[:], EXP, scale=SCALE)
                        if pre_pv is not None and jp < len(pre_pv) \
                                and pre_pv[jp] is not None:
                            for thunk in pre_pv[jp]:
                                thunk()
                        for jb, co in zip(jbs, (0, 512)):
                            nc.tensor.matmul(
                                ppA[:], vn[jb][:, hA * VW:(hA + 1) * VW],
                                eA[:, co:co + 512],
                                start=(jb == 0), stop=(jb == NB - 1))
                        for jb, co in zip(jbs, (0, 512)):
                            nc.tensor.matmul(
                                ppB[:], vn[jb][:, hB * VW:(hB + 1) * VW],
                                eB[:, co:co + 512],
                                start=(jb == 0), stop=(jb == NB - 1))
                    # normalize: out^T = PV[0:64] / rowsum
                    for pp, po in ((ppA, 0), (ppB, 64)):
                        rs = p_nrm.tile([1, 512], F32, tag="rs", name=f"rs{t}{ih}{po}")
                        nc.vector.tensor_copy(rs[:], pp[HD:VW, :])
                        bc = p_nrm.tile([64, 512], F32, tag="bc", name=f"bc{t}{ih}{po}")
                        nc.gpsimd.partition_broadcast(bc[:], rs[:])
                        rc = p_nrm.tile([64, 512], F32, tag="rc", name=f"rc{t}{ih}{po}")
                        nc.vector.reciprocal_approx_fast(rc[:], bc[:])
                        if po == 0:
                            nc.vector.tensor_mul(
                                outT[t][0:64, ih * 512:(ih + 1) * 512],
                                pp[0:HD, :], rc[:])
                        else:
                            ob = p_nrm.tile([64, 512], PROJ_DT, tag="ob", name=f"ob{t}{ih}")
                            nc.vector.tensor_mul(ob[:], pp[0:HD, :], rc[:])
                            nc.sync.dma_start(
                                outT[t][64:128, ih * 512:(ih + 1) * 512], ob[:])

            # ---- proj + bias + store; the last seq block goes out in
            # quarter-width pieces so the final exposed drain+store chain
            # after the last matmul is as short as possible
            for nb in range(NB - 1):
                for cp in range(2):
                    emit_proj_half(nb, cp)
            ys7 = p_y.tile([128, C], BF16, tag="ys", name="ys7")
            for cq in range(4):
                py = ps_mm.tile([128, 192], F32, tag="pmm", name=f"py7{cq}")
                for t2 in range(CB):
                    nc.tensor.matmul(
                        py[:], outT[t2][:, 7 * 128:8 * 128],
                        wp[:, t2, cq * 192:(cq + 1) * 192],
                        start=(t2 == 0), stop=(t2 == CB - 1))
                nc.vector.scalar_tensor_tensor(
                    ys7[:, cq * 192:(cq + 1) * 192], py[:], 1.0,
                    bias_bc[:, cq * 192:(cq + 1) * 192], MUL, ADD)
                nc.sync.dma_start(
                    y.ap()[7 * 128:8 * 128, cq * 192:(cq + 1) * 192],
                    ys7[:, cq * 192:(cq + 1) * 192])

    nc.compile()
    return nc


def _get_nc():
    global _NC
    if _NC is None:
        _NC = _build()
    return _NC


def _run(in_maps, trace=False, tmpdir=None):
    return bass_utils.run_bass_kernel_spmd(
        _get_nc(), in_maps, core_ids=list(range(B)), trace=trace, tmpdir=tmpdir)


def _in_maps(x, w_qkv, w_proj, b_proj):
    import ml_dtypes
    bf = ml_dtypes.bfloat16
    x = np.ascontiguousarray(np.asarray(x, dtype=np.float32).astype(bf))
    w_qkv = np.ascontiguousarray(np.asarray(w_qkv, dtype=np.float32).astype(bf))
    w_proj = np.ascontiguousarray(np.asarray(w_proj, dtype=np.float32).astype(bf))
    b_proj = np.ascontiguousarray(np.asarray(b_proj, dtype=np.float32)).reshape(1, C)
    return [
        {"x": np.ascontiguousarray(x[b]), "w_qkv": w_qkv,
         "w_proj": w_proj, "b_proj": b_proj}
        for b in range(B)
    ]


def kernel(x, w_qkv, w_proj, b_proj):
    res = _run(_in_maps(x, w_qkv, w_proj, b_proj))
    return np.stack([np.asarray(res.results[b]["y"], dtype=np.float32)
                     for b in range(B)], axis=0)


# revision 19
# speedup vs baseline: 1.0204x; 1.0204x over previous
"""Multi-head attention forward (B=8, N=1024, C=768, H=12) on 8 TRN2 NeuronCores.

Sharding: data-parallel over batch — core b computes batch b end-to-end
(weights replicated, no collectives). Per-core dataflow, all matmuls bf16
with fp32 PSUM accumulation:

  x [1024,768] --cast+PE transpose (batched per seq block)--> xT [768,1024]
  qT,kT[t] = w_qkv-stationary matmuls over xT               [feat, seq]
  v        = xT-stationary matmuls over w_qkv[:, 1536:]     [seq, feat(+ones)]
  S^T  = kT-stationary matmuls over qT (2 heads packed in PE row groups)
  E^T  = exp(S^T / 8) via ACT straight from PSUM (no max-sub needed)
  PV   = v_aug-stationary matmuls over E^T -> [out^T ; rowsum] in PSUM
  out^T = PV[0:64] * (1/rowsum)   (gpsimd partition-broadcast + fast recip)
  y = out^T-stationary matmuls over w_proj + bias, stored bf16

Schedule (trace-driven): the PE is the bottleneck engine (~166us busy of
~198us wall; fp8 double-pumping would halve the PV cost but e4m3
quantization of E or v adds ~2.2-3.1e-2 max-rel error vs the 2e-2 budget,
measured in simulation, so everything stays bf16). What remains is keeping
the PE stream dense:
  - 28 dummy identity transposes spin the PE up to full p-state while the
    first x block is still in flight (the PE runs at half clock for ~3us
    after any idle period).
  - x arrives as 8 per-seq-block descriptors on the SP hardware DMA queue,
    depth-4 dependency-chained so blocks complete in order (unchained
    concurrent descriptors all complete striped-at-the-end, which starves
    the transposes); q+k weights ride the ACT queue as 6 concurrent
    full-row descriptors; v weights + w_proj follow on SP. The nh=0 q/k
    matmul groups are woven between transpose batches to cover the x tail.
  - per-seq-block x^T psum is drained by a single DVE copy (bf16 2x mode)
    into one [128, CB, N] tile, keeping ACT free of prologue work.
  - v-computation is woven into the first head-pair's attention between
    exp and PV; the next head-pair's q/k matmuls are woven one psum-group
    per jp into the previous pair's second i-half so the ACT exp queue
    never drains at head boundaries.
  - proj epilogue: ACT drains each py psum half (fast psum recycle while
    DVE finishes the last normalizations), DVE adds the bias in SBUF off
    the recycle path, y is stored bf16 per half-block (host upcasts).
Measured ~197-200us per core (run-to-run thermal throttle noise ~±4us),
output max rel err 8.6e-3 vs the fp32 reference.
"""
import numpy as np
from contextlib import ExitStack

import concourse.bacc as bacc
import concourse.tile as tile
from concourse import mybir, bass_utils, masks
from concourse.tile import add_dep_helper

F32 = mybir.dt.float32
BF16 = mybir.dt.bfloat16
EXP = mybir.ActivationFunctionType.Exp
MUL = mybir.AluOpType.mult
ADD = mybir.AluOpType.add

QKV_DT = BF16
ATT_DT = BF16
PROJ_DT = BF16

B = 8
N = 1024       # sequence length
C = 768        # channels
H = 12         # heads
HD = 64        # head dim
NB = N // 128  # 8 seq blocks
CB = C // 128  # 6 channel chunks
HP = H // 2    # 6 head pairs
VW = HD + 1    # 65: v columns per head incl. ones column
SCALE = float(HD) ** -0.5

_NC = None


def _build():
    nc = bacc.Bacc("TRN2", target_bir_lowering=False, debug=False, num_devices=B)
    x = nc.dram_tensor("x", [N, C], BF16, kind="ExternalInput")
    w_qkv = nc.dram_tensor("w_qkv", [C, 3 * C], BF16, kind="ExternalInput")
    w_proj = nc.dram_tensor("w_proj", [C, C], BF16, kind="ExternalInput")
    b_proj = nc.dram_tensor("b_proj", [1, C], F32, kind="ExternalInput")
    y = nc.dram_tensor("y", [N, C], BF16, kind="ExternalOutput")

    with tile.TileContext(nc) as tc, ExitStack() as ctx:
        const = ctx.enter_context(tc.tile_pool(name="const", bufs=1))
        p_qk = ctx.enter_context(tc.tile_pool(name="p_qk", bufs=1))
        p_v = ctx.enter_context(tc.tile_pool(name="p_v", bufs=1))
        p_out = ctx.enter_context(tc.tile_pool(name="p_out", bufs=1))

        # ---- whole-tensor input tiles. Many concurrent descriptors across
        # BOTH hardware DMA queues (SP + ACT): the queue sprays concurrent
        # descriptors over its DMA engines, so per-queue bandwidth scales
        # with outstanding descriptors. w_qkv loads full 4608B rows per
        # packet (one descriptor per 128-row chunk); x is one descriptor
        # per seq block so transposes start on the first arrival.
        xin = const.tile([128, NB, C], BF16, tag="xin")
        wqkv = const.tile([128, CB, 3 * C], QKV_DT, tag="wqkv")
        wp = const.tile([128, CB, C], PROJ_DT, tag="wp")
        bias_row = const.tile([1, C], F32, tag="bias_row")

        x_r = x.ap().rearrange("(i p) c -> p i c", p=128)
        w_r = w_qkv.ap().rearrange("(s p) f -> p s f", p=128)
        wp_r = w_proj.ap().rearrange("(s p) f -> p s f", p=128)
        # x: SP queue, depth-2 chained so seq blocks complete in order and
        # transposes start immediately (unchained concurrent descriptors all
        # complete striped-at-the-end, which stalls the whole prologue).
        xdma = []
        for ib in range(NB):
            dma = nc.sync.dma_start(xin[:, ib:ib + 1, :], x_r[:, ib:ib + 1, :])
            if ib >= 4:
                add_dep_helper(dma.ins, xdma[ib - 4].ins, sync=True,
                               reason="x dma depth-4 chain")
            xdma.append(dma)
        # q+k band: ACT queue, 6 concurrent descriptors (3072B rows). The
        # ACT stream only issues these 6 and is then free for exp.
        for cc in range(CB):
            nc.scalar.dma_start(wqkv[:, cc:cc + 1, 0:2 * C],
                                w_r[:, cc:cc + 1, 0:2 * C])
        nc.scalar.dma_start(bias_row[:], b_proj.ap())
        # v band + w_proj: SP queue after the x chain.
        for cc in range(CB):
            nc.sync.dma_start(wqkv[:, cc:cc + 1, 2 * C:3 * C],
                              w_r[:, cc:cc + 1, 2 * C:3 * C])
        for cc2 in range(0, CB, 2):
            nc.sync.dma_start(wp[:, cc2:cc2 + 2, :], wp_r[:, cc2:cc2 + 2, :])

        ident_bf = const.tile([128, 128], BF16, tag="ident_bf")
        masks.make_identity(nc, ident_bf[:])
        bias_bc = const.tile([128, C], F32, tag="bias_bc")
        nc.gpsimd.partition_broadcast(bias_bc[:], bias_row[:])
        ones12 = const.tile([128, H], F32, tag="ones12")
        nc.vector.memset(ones12[:], 1.0)

        qT = [p_qk.tile([128, N], ATT_DT, tag=f"qT{t}", name=f"qT{t}") for t in range(HP)]
        kT = [p_qk.tile([128, N], ATT_DT, tag=f"kT{t}", name=f"kT{t}") for t in range(HP)]
        vn = [p_v.tile([128, H * VW], ATT_DT, tag=f"v{ib}", name=f"v{ib}") for ib in range(NB)]
        outT = [p_out.tile([128, N], PROJ_DT, tag=f"outT{t}", name=f"outT{t}") for t in range(HP)]

        with (
            tc.tile_pool(name="p_xT", bufs=1) as p_xT,
            tc.tile_pool(name="p_E", bufs=7) as p_E,
            tc.tile_pool(name="p_nrm", bufs=2) as p_nrm,
            tc.tile_pool(name="p_y", bufs=4) as p_y,
            tc.tile_pool(name="ps_mm", bufs=2, space="PSUM") as ps_mm,
            tc.tile_pool(name="ps_s", bufs=2, space="PSUM") as ps_s,
            tc.tile_pool(name="ps_pv", bufs=1, space="PSUM") as ps_pv,
        ):
            # ---- PE p-state warmup: dummy transposes while the first x
            # block is still in flight. The PE needs ~3us of continuous
            # execution to reach full clock; without this the first real
            # transposes+matmuls run at half speed.
            for wi in range(28):
                warm = ps_mm.tile([128, 128], BF16, tag="pmm", name=f"warm{wi}")
                nc.tensor.transpose(warm[:], ident_bf[:], ident_bf[:])

            # ---- x^T via PE transposes; one DVE copy drains the whole
            # psum block per seq chunk (bf16 2x mode), keeping ACT free
            xT = p_xT.tile([128, CB, N], QKV_DT, tag="xT", name="xT")

            def emit_transpose(ib):
                pt = ps_mm.tile([128, C], BF16, tag="pmm", name=f"ptr{ib}")
                for cc in range(CB):
                    nc.tensor.transpose(pt[:, cc * 128:(cc + 1) * 128],
                                        xin[:, ib, cc * 128:(cc + 1) * 128],
                                        ident_bf[:])
                nc.vector.tensor_copy(
                    xT[:, :, ib * 128:(ib + 1) * 128],
                    pt[:].rearrange("p (s f) -> p s f", f=128))

            def emit_qk_group(t, gi):
                f_off, dst = ((0, qT), (C, kT))[gi // 2]
                nh = gi % 2
                pq = ps_mm.tile([128, 512], F32, tag="pmm", name=f"pq{t}_{gi}")
                for cc in range(CB):
                    nc.tensor.matmul(
                        pq[:],
                        wqkv[:, cc, f_off + t * 128: f_off + (t + 1) * 128],
                        xT[:, cc, nh * 512:(nh + 1) * 512],
                        start=(cc == 0), stop=(cc == CB - 1))
                nc.vector.tensor_copy(dst[t][:, nh * 512:(nh + 1) * 512], pq[:])

            def emit_qk(t):
                for gi in range(4):
                    emit_qk_group(t, gi)

            def emit_v_group(ib, half):
                pv = ps_mm.tile([128, 384], F32, tag="pmm", name=f"pv{ib}_{half}")
                for cc in range(CB):
                    nc.tensor.matmul(
                        pv[:],
                        xT[:, cc, ib * 128:(ib + 1) * 128],
                        wqkv[:, cc, 2 * C + half * 384:2 * C + (half + 1) * 384],
                        start=(cc == 0), stop=(cc == CB - 1))
                nc.vector.tensor_copy(
                    vn[ib][:, half * 6 * VW:(half + 1) * 6 * VW]
                    .rearrange("p (h d) -> p h d", d=VW)[:, :, 0:HD],
                    pv[:].rearrange("p (h d) -> p h d", d=HD))
                if half == 1:
                    nc.vector.tensor_copy(
                        vn[ib][:].rearrange("p (h d) -> p h d", d=VW)[:, :, HD:VW],
                        ones12[:])

            # seq blocks 0-3, then the nh=0 q/k groups (they only need the
            # first half of x^T), then blocks 4-7, then the nh=1 groups —
            # keeps the PE busy while the x dma chain streams in.
            for ib in range(4):
                emit_transpose(ib)
            emit_qk_group(0, 0)
            emit_transpose(4)
            emit_qk_group(0, 2)
            for ib in range(5, NB):
                emit_transpose(ib)
            emit_qk_group(0, 1)
            emit_qk_group(0, 3)
            for ib0 in range(2):
                for h0 in range(2):
                    emit_v_group(ib0, h0)

            # proj half-block: matmul group + ACT drain + DVE bias-add +
            # store (used in the tail, and woven into the last pair's
            # exp-wait slots for nb=0)
            ys_tiles = {}

            def emit_proj_half(nb, cp, last=False):
                if nb not in ys_tiles:
                    ys_tiles[nb] = p_y.tile([128, C], BF16, tag="ys", name=f"ys{nb}")
                ys = ys_tiles[nb]
                py = ps_mm.tile([128, 384], F32, tag="pmm", name=f"py{nb}{cp}")
                for t2 in range(CB):
                    nc.tensor.matmul(
                        py[:], outT[t2][:, nb * 128:(nb + 1) * 128],
                        wp[:, t2, cp * 384:(cp + 1) * 384],
                        start=(t2 == 0), stop=(t2 == CB - 1))
                if last:
                    nc.vector.scalar_tensor_tensor(
                        ys[:, cp * 384:(cp + 1) * 384], py[:], 1.0,
                        bias_bc[:, cp * 384:(cp + 1) * 384], MUL, ADD)
                else:
                    nc.scalar.copy(ys[:, cp * 384:(cp + 1) * 384], py[:])
                    nc.vector.scalar_tensor_tensor(
                        ys[:, cp * 384:(cp + 1) * 384],
                        ys[:, cp * 384:(cp + 1) * 384], 1.0,
                        bias_bc[:, cp * 384:(cp + 1) * 384], MUL, ADD)
                nc.sync.dma_start(
                    y.ap()[nb * 128:(nb + 1) * 128, cp * 384:(cp + 1) * 384],
                    ys[:, cp * 384:(cp + 1) * 384])

            # ---- per head-pair: q^T,k^T then attention (pipelines across t)
            for t in range(HP):
                hA, hB = 2 * t, 2 * t + 1
                for ih in range(2):
                    pre_pv = None
                    if t == 0 and ih == 0:
                        pre_pv = [
                            [lambda ib=ib, h=h: emit_v_group(ib, h)
                             for ib in (2 * jp_ + 2, 2 * jp_ + 3) for h in (0, 1)]
                            for jp_ in range(3)
                        ]
                    elif ih == 1 and t + 1 < HP:
                        # weave next head-pair's q/k matmuls one psum-group per
                        # jp so the ACT exp queue never drains at the boundary
                        pre_pv = [[lambda g=g, tt=t: emit_qk_group(tt + 1, g)]
                                  for g in range(4)]
                    ppA = ps_pv.tile([VW, 512], F32, tag="pvA", name=f"ppA{t}{ih}")
                    ppB = ps_pv.tile([VW, 512], F32, tag="pvB", name=f"ppB{t}{ih}")
                    for jp in range(4):
                        jbs = (2 * jp, 2 * jp + 1)
                        sA = ps_s.tile([128, 1024], F32, tag="s2", name=f"sA{t}{ih}{jp}")
                        sB = ps_s.tile([128, 1024], F32, tag="s2", name=f"sB{t}{ih}{jp}")
                        # batch same-PE-tile-config matmuls: each (0,0) ->
                        # (64,0) row-tile reconfig costs ~90ns on hw
                        for jb, co in zip(jbs, (0, 512)):
                            nc.tensor.matmul(
                                sA[:, co:co + 512],
                                kT[t][0:64, jb * 128:(jb + 1) * 128],
                                qT[t][0:64, ih * 512:(ih + 1) * 512],
                                start=True, stop=True, tile_position=(0, 0))
                        for jb, co in zip(jbs, (0, 512)):
                            nc.tensor.matmul(
                                sB[:, co:co + 512],
                                kT[t][64:128, jb * 128:(jb + 1) * 128],
                                qT[t][64:128, ih * 512:(ih + 1) * 512],
                                start=True, stop=True, tile_position=(64, 0))
                        eA = p_E.tile([128, 1024], ATT_DT, tag="e2", name=f"eA{t}{ih}{jp}")
                        eB = p_E.tile([128, 1024], ATT_DT, tag="e2", name=f"eB{t}{ih}{jp}")
                        nc.scalar.activation(eA[:], sA[:], EXP, scale=SCALE)
                        nc.scalar.activation(eB[:], sB[:], EXP, scale=SCALE)
                        if pre_pv is not None and jp < len(pre_pv) \
                                and pre_pv[jp] is not None:
                            for thunk in pre_pv[jp]:
                                thunk()
                        for jb, co in zip(jbs, (0, 512)):
                            nc.tensor.matmul(
                                ppA[:], vn[jb][:, hA * VW:(hA + 1) * VW],
                                eA[:, co:co + 512],
                                start=(jb == 0), stop=(jb == NB - 1))
                        for jb, co in zip(jbs, (0, 512)):
                            nc.tensor.matmul(
                                ppB[:], vn[jb][:, hB * VW:(hB + 1) * VW],
                                eB[:, co:co + 512],
                                start=(jb == 0), stop=(jb == NB - 1))
                    # normalize: out^T = PV[0:64] / rowsum. For the final
                    # half, head B goes first (its ob-DMA chain gates the
                    # nb>=4 proj blocks) and its ob rides the ACT queue,
                    # skipping the y-store backlog on the SP queue.
                    final_half = (t == HP - 1 and ih == 1)
                    order = ((ppB, 64), (ppA, 0)) if final_half else ((ppA, 0), (ppB, 64))
                    for pp, po in order:
                        rs = p_nrm.tile([1, 512], F32, tag="rs", name=f"rs{t}{ih}{po}")
                        nc.vector.tensor_copy(rs[:], pp[HD:VW, :])
                        bc = p_nrm.tile([64, 512], F32, tag="bc", name=f"bc{t}{ih}{po}")
                        nc.gpsimd.partition_broadcast(bc[:], rs[:])
                        rc = p_nrm.tile([64, 512], F32, tag="rc", name=f"rc{t}{ih}{po}")
                        nc.vector.reciprocal_approx_fast(rc[:], bc[:])
                        if po == 0:
                            nc.vector.tensor_mul(
                                outT[t][0:64, ih * 512:(ih + 1) * 512],
                                pp[0:HD, :], rc[:])
                        else:
                            ob = p_nrm.tile([64, 512], PROJ_DT, tag="ob", name=f"ob{t}{ih}")
                            nc.vector.tensor_mul(ob[:], pp[0:HD, :], rc[:])
                            obeng = nc.scalar if final_half else nc.sync
                            obeng.dma_start(
                                outT[t][64:128, ih * 512:(ih + 1) * 512], ob[:])

            # ---- proj + bias + store; the last seq block goes out in
            # quarter-width pieces so the final exposed drain+store chain
            # after the last matmul is as short as possible
            for nb in range(NB - 1):
                for cp in range(2):
                    emit_proj_half(nb, cp)
            ys7 = p_y.tile([128, C], BF16, tag="ys", name="ys7")
            for cq in range(4):
                py = ps_mm.tile([128, 192], F32, tag="pmm", name=f"py7{cq}")
                for t2 in range(CB):
                    nc.tensor.matmul(
                        py[:], outT[t2][:, 7 * 128:8 * 128],
                        wp[:, t2, cq * 192:(cq + 1) * 192],
                        start=(t2 == 0), stop=(t2 == CB - 1))
                nc.vector.scalar_tensor_tensor(
                    ys7[:, cq * 192:(cq + 1) * 192], py[:], 1.0,
                    bias_bc[:, cq * 192:(cq + 1) * 192], MUL, ADD)
                nc.sync.dma_start(
                    y.ap()[7 * 128:8 * 128, cq * 192:(cq + 1) * 192],
                    ys7[:, cq * 192:(cq + 1) * 192])

    nc.compile()
    return nc


def _get_nc():
    global _NC
    if _NC is None:
        _NC = _build()
    return _NC


def _run(in_maps, trace=False, tmpdir=None):
    return bass_utils.run_bass_kernel_spmd(
        _get_nc(), in_maps, core_ids=list(range(B)), trace=trace, tmpdir=tmpdir)


def _in_maps(x, w_qkv, w_proj, b_proj):
    import ml_dtypes
    bf = ml_dtypes.bfloat16
    x = np.ascontiguousarray(np.asarray(x, dtype=np.float32).astype(bf))
    w_qkv = np.ascontiguousarray(np.asarray(w_qkv, dtype=np.float32).astype(bf))
    w_proj = np.ascontiguousarray(np.asarray(w_proj, dtype=np.float32).astype(bf))
    b_proj = np.ascontiguousarray(np.asarray(b_proj, dtype=np.float32)).reshape(1, C)
    return [
        {"x": np.ascontiguousarray(x[b]), "w_qkv": w_qkv,
         "w_proj": w_proj, "b_proj": b_proj}
        for b in range(B)
    ]


def kernel(x, w_qkv, w_proj, b_proj):
    res = _run(_in_maps(x, w_qkv, w_proj, b_proj))
    return np.stack([np.asarray(res.results[b]["y"], dtype=np.float32)
                     for b in range(B)], axis=0)
